# revision 38
# baseline (speedup 1.0000x reference)
"""Trainium2 Bass kernel for nn_MultiHeadAttention_72370198938219.

Data-parallel over batch: core b computes batch element b end-to-end.
Per-core pipeline (S=2048, D=1024, H=16, DH=64):
  P1: transpose X tiles on TensorE; V projection (f32r matmuls);
      low-rank T1 = X @ W*1 for Q and K.
  P2: per head-pair p (2 heads share a 128-partition tile):
      QpT/KpT = W*2.T @ T1; stage-1 scores via block-diag(A); softmax over
      free dim; attn_VV out; transpose attn; VV = attnT.T @ Vp;
      stage-2 scores via block-diag(A.T); exact softmax over d; transpose;
      VVV; write per-pair VVV to DRAM scratch.
  P3: per row-tile: scramble-read scratch (the faithful torch reshape is a
      row-major reinterpretation, done by the read AP), + residual, LN1,
      transpose, fc with (Wfc + I) folded on host, LN2, store.
"""

import os

import numpy as np

import concourse.bass as bass
import concourse.mybir as mybir
import concourse.tile as tile
from concourse.bass_utils import run_bass_kernel_spmd
from concourse.masks import make_identity
from bass_rust import ScopedClock

B, S, D, H, DH = 8, 2048, 1024, 16, 64
NP = H // 2          # head pairs
SC = S // 128        # s-chunks of 128 rows
DC = D // 128        # d-chunks of 128
F32 = mybir.dt.float32
F32R = mybir.dt.float32r
BF16 = mybir.dt.bfloat16
AF = mybir.ActivationFunctionType
ALU = mybir.AluOpType
EPS = 1e-5


class _TC(tile.TileContext):
    """TileContext with a walrus-compatible kernel tail: this container's
    walrus rejects instructions carrying more than one sync wait, so emit
    the tail-drain waits as standalone EventSemaphore waits instead."""

    def _drain_and_barrier(self, tick_clock, wait_clock):
        probe = self.nc.sync.nop(nofuse=True)
        wait_clock.add_sem_waits(
            probe.ins, ScopedClock({None: tick_clock.global_clock})
        )
        si = probe.ins.sync_info
        waits = list(si.on_wait) if si is not None and si.on_wait else []
        probe.ins.sync_info = mybir.SyncInfo(on_wait=[], on_update=[])
        assert self.sems is not None
        handles = {}
        for h in self.sems.allocated().values():
            handles[getattr(h, "num", None) or getattr(h, "id", None)] = h
        for w in waits:
            self.nc.sync.wait_ge(handles[w.id], w.wait_value)
        drain_inst = self.nc.sync.drain()
        wait_clock.add_sem_waits(
            drain_inst.ins,
            ScopedClock({None: tick_clock.global_clock}),
            ScopedClock({None: tick_clock.global_clock}),
        )
        self.nc.all_engine_barrier()
        popped = self.nc._tile_sem_poison_stack.pop()
        assert popped is self._sem_poison
        self.nc.clear_and_free_semaphores(list(self.sems.allocated().values()))
        self.nc.all_engine_barrier()


def _split_excess_waits(nc):
    """Hoist extra sync waits into standalone EventSemaphore instructions
    (this walrus accepts 1 wait per instruction, 2 on EventSemaphore)."""
    for fn in nc.m.functions:
        for bb in fn.blocks:
            insts = bb.instructions
            out = []
            changed = False
            for inst in insts:
                si = inst.sync_info
                waits = list(si.on_wait) if si is not None and si.on_wait else []
                cap = 2 if isinstance(inst, mybir.InstEventSemaphore) else 1
                if len(waits) > cap:
                    extra = waits[:-cap]
                    for k in range(0, len(extra), 2):
                        out.append(
                            mybir.InstEventSemaphore(
                                name=nc.get_next_instruction_name(),
                                engine=inst.engine,
                                ins=[],
                                outs=[],
                                sync_info=mybir.SyncInfo(
                                    on_wait=extra[k : k + 2], on_update=[]
                                ),
                            )
                        )
                    inst.sync_info = mybir.SyncInfo(
                        on_wait=waits[-cap:], on_update=list(si.on_update or [])
                    )
                    changed = True
                out.append(inst)
            if changed:
                bb.instructions = out


def _build(apply_gb, legalize=True, stage=3, p2sub=9):
    nc = bass.Bass()
    xq = nc.dram_tensor("xq", [S, D], F32, kind="ExternalInput")
    xq_bf = nc.dram_tensor("xq_bf", [S, D], BF16, kind="ExternalInput")
    xk_bf = nc.dram_tensor("xk_bf", [S, D], BF16, kind="ExternalInput")
    xv_bf = nc.dram_tensor("xv_bf", [S, D], BF16, kind="ExternalInput")
    wq1 = nc.dram_tensor("wq1", [D, 32], BF16, kind="ExternalInput")
    wq2 = nc.dram_tensor("wq2", [32, D], F32R, kind="ExternalInput")
    wk1 = nc.dram_tensor("wk1", [D, 32], BF16, kind="ExternalInput")
    wk2 = nc.dram_tensor("wk2", [32, D], F32R, kind="ExternalInput")
    wv = nc.dram_tensor("wv", [D, D], BF16, kind="ExternalInput")
    wfc = nc.dram_tensor("wfc", [D, D], F32R, kind="ExternalInput")  # Wfc + I
    abd = nc.dram_tensor("abd", [128, 128], F32R, kind="ExternalInput")
    atbd = nc.dram_tensor("atbd", [128, 128], F32R, kind="ExternalInput")
    lng = nc.dram_tensor("lng", [D], F32, kind="ExternalInput")
    lnb = nc.dram_tensor("lnb", [D], F32, kind="ExternalInput")
    out = nc.dram_tensor("out", [S, D], F32, kind="ExternalOutput")
    attn = nc.dram_tensor("attn", [H, DH, S], F32, kind="ExternalOutput")

    with _TC(nc) as tc:
        with (
            tc.tile_pool(name="const", bufs=1) as constp,
            tc.tile_pool(name="vp", bufs=1) as vpp,
            tc.tile_pool(name="t1", bufs=1) as t1p,
            tc.tile_pool(name="wfcp", bufs=1) as wfcp,
            tc.tile_pool(name="dram", bufs=1, space="DRAM") as dramp,
        ):
            ident = constp.tile([128, 128], F32)
            make_identity(nc, ident)
            ident_r = constp.tile([128, 128], F32R)
            nc.scalar.copy(out=ident_r[:], in_=ident[:])
            ident_bf = constp.tile([128, 128], BF16)
            nc.scalar.copy(out=ident_bf[:], in_=ident[:])
            abd_sb = constp.tile([128, 128], F32R)
            nc.sync.dma_start(out=abd_sb[:], in_=abd[:])
            atbd_sb = constp.tile([128, 128], F32R)
            nc.sync.dma_start(out=atbd_sb[:], in_=atbd[:])
            wq1_sb = constp.tile([128, DC, 32], BF16)
            nc.sync.dma_start(out=wq1_sb[:], in_=wq1.rearrange("(c p) e -> p c e", p=128))
            wk1_sb = constp.tile([128, DC, 32], BF16)
            nc.sync.dma_start(out=wk1_sb[:], in_=wk1.rearrange("(c p) e -> p c e", p=128))
            wq2_sb = constp.tile([32, D], F32R)
            nc.sync.dma_start(out=wq2_sb[:], in_=wq2[:])
            wk2_sb = constp.tile([32, D], F32R)
            nc.sync.dma_start(out=wk2_sb[:], in_=wk2[:])
            eps_sb = constp.tile([128, 1], F32)
            nc.vector.memset(eps_sb, EPS)
            if apply_gb:
                g_sb = constp.tile([128, D], F32)
                nc.sync.dma_start(
                    out=g_sb[:],
                    in_=bass.AP(tensor=lng.tensor, offset=lng.offset,
                                ap=[[0, 128], [1, D]]),
                )
                b_sb = constp.tile([128, D], F32)
                nc.sync.dma_start(
                    out=b_sb[:],
                    in_=bass.AP(tensor=lnb.tensor, offset=lnb.offset,
                                ap=[[0, 128], [1, D]]),
                )

            vp_sb = vpp.tile([128, SC, D], BF16)       # Vp[s,e], s-chunked
            t1q_sb = t1p.tile([32, S], F32R)           # (Xq@Wq1).T
            t1k_sb = t1p.tile([32, S], F32R)
            # Per-head scratch in out1_pre ("scrambled") row layout, prefilled
            # with the Xq residual rows; the scatter-write ACCUMULATES VVV into
            # it with the faithful-torch-reshape AP, so P3 reads VVV+residual.
            scratch_h = []
            for h in range(H):
                s_t = dramp.tile([128, D], F32, tag=f"scr{h}", name=f"scr{h}")
                nc.sync.dma_start(out=s_t[:], in_=xq[h * 128 : (h + 1) * 128, :])
                scratch_h.append(s_t)

            # ---------------- P1: xbar-transposed loads + projections ----------------
            with (
                tc.tile_pool(name="wvp", bufs=1) as wvp,
                tc.tile_pool(name="p1t", bufs=1) as p1t,
                tc.tile_pool(name="psA", bufs=3, space="PSUM") as psA,
                tc.tile_pool(name="psT1", bufs=2, space="PSUM") as psT1,
            ):
                wv_sb = wvp.tile([128, DC, D], BF16)
                nc.sync.dma_start(out=wv_sb[:], in_=wv.rearrange("(c p) e -> p c e", p=128))
                xts = []
                for name, srcd in (("xvt", xv_bf), ("xqt", xq_bf), ("xkt", xk_bf)):
                    xt = p1t.tile([128, DC, S], BF16, tag=name, name=name)
                    for dc in range(DC):
                        nc.sync.dma_start(
                            out=xt[:, dc, :],
                            in_=srcd[:, dc * 128 : (dc + 1) * 128],
                            transpose=True,
                        )
                    xts.append(xt)
                xvt, xqt, xkt = xts
                for jj in range(4):  # s-blocks of 512
                    sb0 = jj * 512
                    for sub in range(4):
                        s0 = sb0 + sub * 128
                        pv0 = psA.tile([128, 512], F32, tag="A", name="pv0")
                        pv1 = psA.tile([128, 512], F32, tag="A", name="pv1")
                        for dc in range(DC):
                            for fc, pv in ((0, pv0), (1, pv1)):
                                nc.tensor.matmul(
                                    pv[:],
                                    xvt[:, dc, s0 : s0 + 128],
                                    wv_sb[:, dc, fc * 512 : (fc + 1) * 512],
                                    start=(dc == 0),
                                    stop=(dc == DC - 1),
                                )
                        for fc, pv in ((0, pv0), (1, pv1)):
                            nc.vector.tensor_copy(
                                out=vp_sb[:, jj * 4 + sub, fc * 512 : (fc + 1) * 512],
                                in_=pv[:],
                            )
                    t1q_ps = psT1.tile([32, 512], F32, tag="t1q")
                    t1k_ps = psT1.tile([32, 512], F32, tag="t1k")
                    for dc in range(DC):
                        nc.tensor.matmul(
                            t1q_ps[:],
                            wq1_sb[:, dc, :],
                            xqt[:, dc, sb0 : sb0 + 512],
                            start=(dc == 0),
                            stop=(dc == DC - 1),
                        )
                        nc.tensor.matmul(
                            t1k_ps[:],
                            wk1_sb[:, dc, :],
                            xkt[:, dc, sb0 : sb0 + 512],
                            start=(dc == 0),
                            stop=(dc == DC - 1),
                        )
                    nc.scalar.copy(out=t1q_sb[:, sb0 : sb0 + 512], in_=t1q_ps[:])
                    nc.scalar.copy(out=t1k_sb[:, sb0 : sb0 + 512], in_=t1k_ps[:])

            wfc_sb = wfcp.tile([128, DC, D], F32R)
            nc.sync.dma_start(out=wfc_sb[:], in_=wfc.rearrange("(c p) e -> p c e", p=128))
            # ---------------- P2: per head-pair attention ----------------
            if stage >= 2:
              with (
                tc.tile_pool(name="p2qk", bufs=2) as p2qk,
                tc.tile_pool(name="p2a", bufs=2) as p2a,
                tc.tile_pool(name="p2b", bufs=2) as p2b,
                tc.tile_pool(name="psA2", bufs=3, space="PSUM") as psA2,
                tc.tile_pool(name="psB", bufs=2, space="PSUM") as psB,
            ):
                for p in range(NP):
                    qpt = p2qk.tile([128, S], F32R, tag="qpt")
                    kpt = p2qk.tile([128, S], F32R, tag="kpt")
                    for wsb, dst in ((wq2_sb, qpt), (wk2_sb, kpt)):
                        tsb = t1q_sb if dst is qpt else t1k_sb
                        for blk in range(4):
                            pq = psA2.tile([128, 512], F32, tag="A2", name="pq")
                            nc.tensor.matmul(
                                pq[:],
                                wsb[:, p * 128 : (p + 1) * 128],
                                tsb[:, blk * 512 : (blk + 1) * 512],
                                start=True, stop=True,
                            )
                            nc.scalar.copy(out=dst[:, blk * 512 : (blk + 1) * 512], in_=pq[:])

                    if p2sub < 2:
                        continue
                    # stage 1: scores1[d2, s] = blockdiag(A).T-style product
                    attn1 = p2a.tile([128, S], F32, tag="attn1")
                    zpart = p2a.tile([128, 4], F32, tag="zpart")
                    for blk in range(4):
                        ps1 = psA2.tile([128, 512], F32, tag="A2")
                        nc.tensor.matmul(
                            ps1[:],
                            abd_sb[:],
                            kpt[:, blk * 512 : (blk + 1) * 512],
                            start=True, stop=True,
                        )
                        nc.scalar.activation(
                            out=attn1[:, blk * 512 : (blk + 1) * 512],
                            in_=ps1[:],
                            func=AF.Exp,
                            scale=0.125,
                            accum_out=zpart[:, blk : blk + 1],
                        )
                    z1 = p2a.tile([128, 1], F32, tag="z1")
                    nc.vector.reduce_sum(z1[:], zpart[:], axis=mybir.AxisListType.X)
                    nc.vector.reciprocal(z1[:], z1[:])
                    attn1n = attn1
                    nc.vector.tensor_scalar_mul(attn1n[:], attn1[:], z1[:])
                    nc.sync.dma_start(
                        out=attn[2 * p : 2 * p + 2].rearrange("h d s -> (h d) s"),
                        in_=attn1n[:],
                    )
                    if p2sub < 3:
                        continue
                    # transpose attn1n -> [s, d2] chunks (bf16); VV accumulation
                    attn1b = p2a.tile([128, S], BF16, tag="attn1b")
                    nc.vector.tensor_copy(out=attn1b[:], in_=attn1n[:])
                    attn1t = p2a.tile([128, SC, 128], BF16, tag="attn1t")
                    for g in range(4):
                        ptb_f = psA2.tile([128, 512], F32, tag="A2", name="ptb")
                        ptb = ptb_f.bitcast(BF16)[:, 0:512]
                        for c in range(4):
                            ch = g * 4 + c
                            nc.tensor.transpose(
                                ptb[:, c * 128 : (c + 1) * 128],
                                attn1b[:, ch * 128 : (ch + 1) * 128],
                                ident_bf[:],
                            )
                        nc.scalar.copy(
                            out=attn1t[:, g * 4 : (g + 1) * 4, :].rearrange("p c f -> p (c f)"),
                            in_=ptb[:],
                        )
                    pvv = psA2.tile([128, 512], F32, tag="A2")
                    for c in range(SC):
                        nc.tensor.matmul(
                            pvv[:, 0:128],
                            attn1t[:, c, :],
                            vp_sb[:, c, p * 128 : (p + 1) * 128],
                            start=(c == 0),
                            stop=(c == SC - 1),
                        )
                    vv = p2a.tile([128, 128], BF16, tag="vv")
                    # zero via DVE (f32r output = legal "rounded" producer)
                    nc.vector.tensor_scalar_mul(vv[:], ident[:], 0.0)
                    nc.scalar.copy(out=vv[0:64, 0:64], in_=pvv[0:64, 0:64])
                    nc.scalar.copy(out=vv[64:128, 64:128], in_=pvv[64:128, 64:128])

                    if p2sub < 4:
                        continue
                    # stage 2, groups of 8 s-chunks
                    vvv = p2b.tile([128, SC, 128], F32, tag="vvv")
                    for g in range(2):
                        G = 8
                        ps2 = psB.tile([128, 1024], F32, tag="B")
                        for c in range(G):
                            ch = g * G + c
                            nc.tensor.matmul(
                                ps2[:, c * 128 : (c + 1) * 128],
                                qpt[:, ch * 128 : (ch + 1) * 128],
                                atbd_sb[:],
                                start=True, stop=True,
                            )
                        ps2v = ps2.rearrange("p (c h f) -> p (c h) f", h=2, f=64)
                        m2 = p2b.tile([128, G * 2], F32, tag="m2")
                        nc.vector.reduce_max(m2[:], ps2v, axis=mybir.AxisListType.X)
                        nc.vector.tensor_tensor(
                            ps2v,
                            ps2v,
                            m2[:, :, None].to_broadcast([128, G * 2, 64]),
                            ALU.subtract,
                        )
                        exps2 = p2b.tile([128, G, 128], BF16, tag="exps2")
                        nc.scalar.activation(
                            out=exps2.rearrange("p c f -> p (c f)"),
                            in_=ps2.rearrange("p f -> p f"),
                            func=AF.Exp,
                            scale=0.125,
                        )
                        z2 = p2b.tile([128, G * 2], F32, tag="z2")
                        nc.vector.reduce_sum(
                            z2[:],
                            exps2.rearrange("p c (h f) -> p (c h) f", h=2, f=64),
                            axis=mybir.AxisListType.X,
                        )
                        nc.vector.reciprocal(z2[:], z2[:])
                        if p2sub < 5:
                            continue
                        att2t = p2b.tile([128, G, 128], BF16, tag="att2t")
                        for gg in range(2):
                            pt2_f = psA2.tile([128, 512], F32, tag="A2", name="pt2")
                            pt2 = pt2_f.bitcast(BF16)[:, 0:512]
                            for c in range(4):
                                ch = gg * 4 + c
                                nc.tensor.transpose(
                                    pt2[:, c * 128 : (c + 1) * 128],
                                    exps2[:, ch, :],
                                    ident_bf[:],
                                )
                            nc.scalar.copy(
                                out=att2t[:, gg * 4 : (gg + 1) * 4, :].rearrange("p c f -> p (c f)"),
                                in_=pt2[:],
                            )
                        if p2sub < 6:
                            continue
                        pvvv = psB.tile([128, 1024], F32, tag="B")
                        for c in range(G):
                            nc.tensor.matmul(
                                pvvv[:, c * 128 : (c + 1) * 128],
                                att2t[:, c, :],
                                vv[:],
                                start=True, stop=True,
                            )
                        nc.vector.tensor_tensor(
                            vvv[:, g * G : (g + 1) * G, :].rearrange(
                                "p c (h f) -> p (c h) f", h=2, f=64
                            ),
                            pvvv.rearrange("p (c h f) -> p (c h) f", h=2, f=64),
                            z2[:, :, None].to_broadcast([128, G * 2, 64]),
                            ALU.mult,
                        )
                    if p2sub < 7:
                        continue
                    # scatter-accumulate with the torch-reshape AP: element
                    # (part=ph*16+pl, chunk c, hh, f) -> row c*8+ph, col pl*64+f
                    for hh in range(2):
                        scrw = scratch_h[2 * p + hh].rearrange(
                            "(c ph) (pl f) -> ph pl c f", c=16, ph=8, pl=16, f=64
                        )
                        nc.gpsimd.dma_start(
                            out=scrw,
                            in_=vvv[:, :, hh * 64 : (hh + 1) * 64],
                            accum_op=ALU.add,
                        )

            # ---------------- P3: reshape + LN + fc + LN ----------------
            if stage >= 3:
              with (
                tc.tile_pool(name="p3", bufs=3) as p3,
                tc.tile_pool(name="p3s", bufs=3) as p3s,
                tc.tile_pool(name="psA3", bufs=4, space="PSUM") as psA3,
            ):
                for t in range(SC):
                    x1 = p3.tile([128, D], F32, tag="x1")
                    nc.sync.dma_start(out=x1[:], in_=scratch_h[t][:])

                    def layer_norm(dst, src, dst_dt):
                        stats = p3s.tile([128, 2, 6], F32, tag="stats")
                        srcv = src.rearrange("p (n f) -> p n f", f=512)
                        nc.vector.bn_stats(out=stats[:, 0, :], in_=srcv[:, 0, :])
                        nc.vector.bn_stats(out=stats[:, 1, :], in_=srcv[:, 1, :])
                        mv = p3s.tile([128, 2], F32, tag="mv")
                        nc.vector.bn_aggr(out=mv[:], in_=stats[:])
                        sd = p3s.tile([128, 1], F32, tag="sd")
                        nc.scalar.activation(
                            out=sd[:], in_=mv[:, 1:2], func=AF.Sqrt, bias=eps_sb[:]
                        )
                        nc.vector.reciprocal(sd[:], sd[:])
                        nmr = p3s.tile([128, 1], F32, tag="nmr")
                        nc.vector.tensor_tensor(nmr[:], mv[:, 0:1], sd[:], ALU.mult)
                        nc.vector.tensor_scalar_mul(nmr[:], nmr[:], -1.0)
                        nc.scalar.activation(
                            out=dst, in_=src, func=AF.Identity, bias=nmr[:], scale=sd[:]
                        )
                        if apply_gb:
                            nc.vector.tensor_tensor(dst, dst, g_sb[:], ALU.mult)
                            nc.vector.tensor_tensor(dst, dst, b_sb[:], ALU.add)

                    o1_dt = F32 if apply_gb else F32R
                    o1 = p3.tile([128, D], o1_dt, tag="o1")
                    layer_norm(o1[:], x1[:], o1_dt)
                    o1t = p3.tile([128, DC, 128], F32R, tag="o1t")
                    t_id = ident if apply_gb else ident_r
                    for g in range(2):
                        pt3_f = psA3.tile([128, 512], F32, tag="A3", name="pt3")
                        pt3 = pt3_f if apply_gb else pt3_f.bitcast(F32R)
                        for c in range(4):
                            dc = g * 4 + c
                            nc.tensor.transpose(
                                pt3[:, c * 128 : (c + 1) * 128],
                                o1[:, dc * 128 : (dc + 1) * 128],
                                t_id[:],
                            )
                        nc.scalar.copy(
                            out=o1t[:, g * 4 : (g + 1) * 4, :].rearrange("p c f -> p (c f)"),
                            in_=pt3[:],
                        )
                    pf0 = psA3.tile([128, 512], F32, tag="A3", name="pf0")
                    pf1 = psA3.tile([128, 512], F32, tag="A3", name="pf1")
                    for dc in range(DC):
                        for pf, fcc in ((pf0, 0), (pf1, 1)):
                            nc.tensor.matmul(
                                pf[:],
                                o1t[:, dc, :],
                                wfc_sb[:, dc, fcc * 512 : (fcc + 1) * 512],
                                start=(dc == 0),
                                stop=(dc == DC - 1),
                            )
                    ot = p3.tile([128, D], F32, tag="ot")
                    stats2 = p3s.tile([128, 2, 6], F32, tag="stats2")
                    nc.vector.bn_stats(out=stats2[:, 0, :], in_=pf0[:])
                    nc.vector.bn_stats(out=stats2[:, 1, :], in_=pf1[:])
                    mv2 = p3s.tile([128, 2], F32, tag="mv2")
                    nc.vector.bn_aggr(out=mv2[:], in_=stats2[:])
                    sd2 = p3s.tile([128, 1], F32, tag="sd2")
                    nc.scalar.activation(
                        out=sd2[:], in_=mv2[:, 1:2], func=AF.Sqrt, bias=eps_sb[:]
                    )
                    nc.vector.reciprocal(sd2[:], sd2[:])
                    nmr2 = p3s.tile([128, 1], F32, tag="nmr2")
                    nc.vector.tensor_tensor(nmr2[:], mv2[:, 0:1], sd2[:], ALU.mult)
                    nc.vector.tensor_scalar_mul(nmr2[:], nmr2[:], -1.0)
                    for pf, fcc in ((pf0, 0), (pf1, 1)):
                        nc.scalar.activation(
                            out=ot[:, fcc * 512 : (fcc + 1) * 512],
                            in_=pf[:],
                            func=AF.Identity,
                            bias=nmr2[:],
                            scale=sd2[:],
                        )
                    if apply_gb:
                        nc.vector.tensor_tensor(ot[:], ot[:], g_sb[:], ALU.mult)
                        nc.vector.tensor_tensor(ot[:], ot[:], b_sb[:], ALU.add)
                    nc.sync.dma_start(out=out[t * 128 : (t + 1) * 128, :], in_=ot[:])

    if legalize:
        _split_excess_waits(nc)
    return nc


_CACHE = {}


def _get_nc(apply_gb):
    if apply_gb not in _CACHE:
        _CACHE[apply_gb] = _build(apply_gb)
    return _CACHE[apply_gb]


last_results = None


def kernel(input_Q, input_K, input_V, Wq1, Wq2, Wk1, Wk2, Wv, Wfc, A, ln_g, ln_b):
    global last_results
    input_Q = np.ascontiguousarray(np.asarray(input_Q, dtype=np.float32))
    input_K = np.ascontiguousarray(np.asarray(input_K, dtype=np.float32))
    input_V = np.ascontiguousarray(np.asarray(input_V, dtype=np.float32))
    A = np.asarray(A, dtype=np.float32)
    ln_g = np.asarray(ln_g, dtype=np.float32)
    ln_b = np.asarray(ln_b, dtype=np.float32)

    abd = np.zeros((128, 128), dtype=np.float32)
    abd[0:64, 0:64] = A
    abd[64:128, 64:128] = A
    atbd = np.zeros((128, 128), dtype=np.float32)
    atbd[0:64, 0:64] = A.T
    atbd[64:128, 64:128] = A.T
    wfcp = (np.asarray(Wfc, dtype=np.float32)
            + np.eye(D, dtype=np.float32)).astype(np.float32)

    apply_gb = not (np.all(ln_g == 1.0) and np.all(ln_b == 0.0))
    nc = _get_nc(apply_gb)

    import ml_dtypes
    shared = {
        "wq1": np.ascontiguousarray(np.asarray(Wq1, dtype=np.float32).astype(ml_dtypes.bfloat16)),
        "wq2": np.ascontiguousarray(Wq2, dtype=np.float32),
        "wk1": np.ascontiguousarray(np.asarray(Wk1, dtype=np.float32).astype(ml_dtypes.bfloat16)),
        "wk2": np.ascontiguousarray(Wk2, dtype=np.float32),
        "wv": np.ascontiguousarray(np.asarray(Wv, dtype=np.float32).astype(ml_dtypes.bfloat16)),
        "wfc": wfcp,
        "abd": abd,
        "atbd": atbd,
        "lng": ln_g,
        "lnb": ln_b,
    }
    in_maps = []
    for b in range(B):
        m = dict(shared)
        m["xq"] = input_Q[b]
        m["xq_bf"] = np.ascontiguousarray(input_Q[b].astype(ml_dtypes.bfloat16))
        m["xk_bf"] = np.ascontiguousarray(input_K[b].astype(ml_dtypes.bfloat16))
        m["xv_bf"] = np.ascontiguousarray(input_V[b].astype(ml_dtypes.bfloat16))
        in_maps.append(m)

    trace = os.environ.get("KERNEL_TRACE") not in (None, "", "0")
    kwargs = {}
    if trace:
        import tempfile
        kwargs = dict(trace=True, tmpdir=tempfile.mkdtemp())
    res = run_bass_kernel_spmd(nc, in_maps, core_ids=list(range(B)), **kwargs)
    last_results = res
    outputs = np.stack([r["out"] for r in res.results])
    attn_vv = np.stack([r["attn"] for r in res.results])
    return outputs, attn_vv


# revision 39
# speedup vs baseline: 1.0293x; 1.0293x over previous
"""Trainium2 Bass kernel for nn_MultiHeadAttention_72370198938219.

Data-parallel over batch: core b computes batch element b end-to-end.
Per-core pipeline (S=2048, D=1024, H=16, DH=64):
  P1: transpose X tiles on TensorE; V projection (f32r matmuls);
      low-rank T1 = X @ W*1 for Q and K.
  P2: per head-pair p (2 heads share a 128-partition tile):
      QpT/KpT = W*2.T @ T1; stage-1 scores via block-diag(A); softmax over
      free dim; attn_VV out; transpose attn; VV = attnT.T @ Vp;
      stage-2 scores via block-diag(A.T); exact softmax over d; transpose;
      VVV; write per-pair VVV to DRAM scratch.
  P3: per row-tile: scramble-read scratch (the faithful torch reshape is a
      row-major reinterpretation, done by the read AP), + residual, LN1,
      transpose, fc with (Wfc + I) folded on host, LN2, store.
"""

import os

import numpy as np

import concourse.bass as bass
import concourse.mybir as mybir
import concourse.tile as tile
from concourse.bass_utils import run_bass_kernel_spmd
from concourse.masks import make_identity
from bass_rust import ScopedClock

B, S, D, H, DH = 8, 2048, 1024, 16, 64
NP = H // 2          # head pairs
SC = S // 128        # s-chunks of 128 rows
DC = D // 128        # d-chunks of 128
F32 = mybir.dt.float32
F32R = mybir.dt.float32r
BF16 = mybir.dt.bfloat16
AF = mybir.ActivationFunctionType
ALU = mybir.AluOpType
EPS = 1e-5


class _TC(tile.TileContext):
    """TileContext with a walrus-compatible kernel tail: this container's
    walrus rejects instructions carrying more than one sync wait, so emit
    the tail-drain waits as standalone EventSemaphore waits instead."""

    def _drain_and_barrier(self, tick_clock, wait_clock):
        probe = self.nc.sync.nop(nofuse=True)
        wait_clock.add_sem_waits(
            probe.ins, ScopedClock({None: tick_clock.global_clock})
        )
        si = probe.ins.sync_info
        waits = list(si.on_wait) if si is not None and si.on_wait else []
        probe.ins.sync_info = mybir.SyncInfo(on_wait=[], on_update=[])
        assert self.sems is not None
        handles = {}
        for h in self.sems.allocated().values():
            handles[getattr(h, "num", None) or getattr(h, "id", None)] = h
        for w in waits:
            self.nc.sync.wait_ge(handles[w.id], w.wait_value)
        drain_inst = self.nc.sync.drain()
        wait_clock.add_sem_waits(
            drain_inst.ins,
            ScopedClock({None: tick_clock.global_clock}),
            ScopedClock({None: tick_clock.global_clock}),
        )
        self.nc.all_engine_barrier()
        popped = self.nc._tile_sem_poison_stack.pop()
        assert popped is self._sem_poison
        self.nc.clear_and_free_semaphores(list(self.sems.allocated().values()))
        self.nc.all_engine_barrier()


def _split_excess_waits(nc):
    """Hoist extra sync waits into standalone EventSemaphore instructions
    (this walrus accepts 1 wait per instruction, 2 on EventSemaphore)."""
    for fn in nc.m.functions:
        for bb in fn.blocks:
            insts = bb.instructions
            out = []
            changed = False
            for inst in insts:
                si = inst.sync_info
                waits = list(si.on_wait) if si is not None and si.on_wait else []
                cap = 2 if isinstance(inst, mybir.InstEventSemaphore) else 1
                if len(waits) > cap:
                    extra = waits[:-cap]
                    for k in range(0, len(extra), 2):
                        out.append(
                            mybir.InstEventSemaphore(
                                name=nc.get_next_instruction_name(),
                                engine=inst.engine,
                                ins=[],
                                outs=[],
                                sync_info=mybir.SyncInfo(
                                    on_wait=extra[k : k + 2], on_update=[]
                                ),
                            )
                        )
                    inst.sync_info = mybir.SyncInfo(
                        on_wait=waits[-cap:], on_update=list(si.on_update or [])
                    )
                    changed = True
                out.append(inst)
            if changed:
                bb.instructions = out


def _build(apply_gb, legalize=True, stage=3, p2sub=9):
    nc = bass.Bass()
    xq = nc.dram_tensor("xq", [S, D], F32, kind="ExternalInput")
    xq_bf = nc.dram_tensor("xq_bf", [S, D], BF16, kind="ExternalInput")
    xk_bf = nc.dram_tensor("xk_bf", [S, D], BF16, kind="ExternalInput")
    xv_bf = nc.dram_tensor("xv_bf", [S, D], BF16, kind="ExternalInput")
    wq1 = nc.dram_tensor("wq1", [D, 32], BF16, kind="ExternalInput")
    wq2 = nc.dram_tensor("wq2", [32, D], F32R, kind="ExternalInput")
    wk1 = nc.dram_tensor("wk1", [D, 32], BF16, kind="ExternalInput")
    wk2 = nc.dram_tensor("wk2", [32, D], F32R, kind="ExternalInput")
    wv = nc.dram_tensor("wv", [D, D], BF16, kind="ExternalInput")
    wfc = nc.dram_tensor("wfc", [D, D], F32R, kind="ExternalInput")  # Wfc + I
    abd = nc.dram_tensor("abd", [128, 128], F32R, kind="ExternalInput")
    atbd = nc.dram_tensor("atbd", [128, 128], F32R, kind="ExternalInput")
    lng = nc.dram_tensor("lng", [D], F32, kind="ExternalInput")
    lnb = nc.dram_tensor("lnb", [D], F32, kind="ExternalInput")
    out = nc.dram_tensor("out", [S, D], F32, kind="ExternalOutput")
    attn = nc.dram_tensor("attn", [H, DH, S], F32, kind="ExternalOutput")

    with _TC(nc) as tc:
        with (
            tc.tile_pool(name="const", bufs=1) as constp,
            tc.tile_pool(name="vp", bufs=1) as vpp,
            tc.tile_pool(name="t1", bufs=1) as t1p,
            tc.tile_pool(name="wfcp", bufs=1) as wfcp,
            tc.tile_pool(name="dram", bufs=1, space="DRAM") as dramp,
        ):
            ident = constp.tile([128, 128], F32)
            make_identity(nc, ident)
            ident_r = constp.tile([128, 128], F32R)
            nc.scalar.copy(out=ident_r[:], in_=ident[:])
            ident_bf = constp.tile([128, 128], BF16)
            nc.scalar.copy(out=ident_bf[:], in_=ident[:])
            abd_sb = constp.tile([128, 128], F32R)
            nc.sync.dma_start(out=abd_sb[:], in_=abd[:])
            atbd_sb = constp.tile([128, 128], F32R)
            nc.sync.dma_start(out=atbd_sb[:], in_=atbd[:])
            wq1_sb = constp.tile([128, DC, 32], BF16)
            nc.sync.dma_start(out=wq1_sb[:], in_=wq1.rearrange("(c p) e -> p c e", p=128))
            wk1_sb = constp.tile([128, DC, 32], BF16)
            nc.sync.dma_start(out=wk1_sb[:], in_=wk1.rearrange("(c p) e -> p c e", p=128))
            wq2_sb = constp.tile([32, D], F32R)
            nc.sync.dma_start(out=wq2_sb[:], in_=wq2[:])
            wk2_sb = constp.tile([32, D], F32R)
            nc.sync.dma_start(out=wk2_sb[:], in_=wk2[:])
            eps_sb = constp.tile([128, 1], F32)
            nc.vector.memset(eps_sb, EPS)
            if apply_gb:
                g_sb = constp.tile([128, D], F32)
                nc.sync.dma_start(
                    out=g_sb[:],
                    in_=bass.AP(tensor=lng.tensor, offset=lng.offset,
                                ap=[[0, 128], [1, D]]),
                )
                b_sb = constp.tile([128, D], F32)
                nc.sync.dma_start(
                    out=b_sb[:],
                    in_=bass.AP(tensor=lnb.tensor, offset=lnb.offset,
                                ap=[[0, 128], [1, D]]),
                )

            vp_sb = vpp.tile([128, SC, D], BF16)       # Vp[s,e], s-chunked
            t1q_sb = t1p.tile([32, S], F32R)           # (Xq@Wq1).T
            t1k_sb = t1p.tile([32, S], F32R)
            # Per-head scratch in out1_pre ("scrambled") row layout, prefilled
            # with the Xq residual rows; the scatter-write ACCUMULATES VVV into
            # it with the faithful-torch-reshape AP, so P3 reads VVV+residual.
            scratch_h = []
            for h in range(H):
                s_t = dramp.tile([128, D], F32, tag=f"scr{h}", name=f"scr{h}")
                nc.sync.dma_start(out=s_t[:], in_=xq[h * 128 : (h + 1) * 128, :])
                scratch_h.append(s_t)

            # ---------------- P1: xbar-transposed loads + projections ----------------
            with (
                tc.tile_pool(name="wvp", bufs=1) as wvp,
                tc.tile_pool(name="p1t", bufs=1) as p1t,
                tc.tile_pool(name="psA", bufs=3, space="PSUM") as psA,
                tc.tile_pool(name="psT1", bufs=2, space="PSUM") as psT1,
            ):
                wv_sb = wvp.tile([128, DC, D], BF16)
                nc.sync.dma_start(out=wv_sb[:], in_=wv.rearrange("(c p) e -> p c e", p=128))
                xts = []
                for name, srcd in (("xvt", xv_bf), ("xqt", xq_bf), ("xkt", xk_bf)):
                    xt = p1t.tile([128, DC, S], BF16, tag=name, name=name)
                    for dc in range(DC):
                        nc.sync.dma_start(
                            out=xt[:, dc, :],
                            in_=srcd[:, dc * 128 : (dc + 1) * 128],
                            transpose=True,
                        )
                    xts.append(xt)
                xvt, xqt, xkt = xts
                for jj in range(4):  # s-blocks of 512
                    sb0 = jj * 512
                    for sub in range(4):
                        s0 = sb0 + sub * 128
                        pv0 = psA.tile([128, 512], F32, tag="A", name="pv0")
                        pv1 = psA.tile([128, 512], F32, tag="A", name="pv1")
                        for dc in range(DC):
                            for fc, pv in ((0, pv0), (1, pv1)):
                                nc.tensor.matmul(
                                    pv[:],
                                    xvt[:, dc, s0 : s0 + 128],
                                    wv_sb[:, dc, fc * 512 : (fc + 1) * 512],
                                    start=(dc == 0),
                                    stop=(dc == DC - 1),
                                )
                        for fc, pv in ((0, pv0), (1, pv1)):
                            nc.vector.tensor_copy(
                                out=vp_sb[:, jj * 4 + sub, fc * 512 : (fc + 1) * 512],
                                in_=pv[:],
                            )
                    t1q_ps = psT1.tile([32, 512], F32, tag="t1q")
                    t1k_ps = psT1.tile([32, 512], F32, tag="t1k")
                    for dc in range(DC):
                        nc.tensor.matmul(
                            t1q_ps[:],
                            wq1_sb[:, dc, :],
                            xqt[:, dc, sb0 : sb0 + 512],
                            start=(dc == 0),
                            stop=(dc == DC - 1),
                        )
                        nc.tensor.matmul(
                            t1k_ps[:],
                            wk1_sb[:, dc, :],
                            xkt[:, dc, sb0 : sb0 + 512],
                            start=(dc == 0),
                            stop=(dc == DC - 1),
                        )
                    nc.scalar.copy(out=t1q_sb[:, sb0 : sb0 + 512], in_=t1q_ps[:])
                    nc.scalar.copy(out=t1k_sb[:, sb0 : sb0 + 512], in_=t1k_ps[:])

            wfc_sb = wfcp.tile([128, DC, D], F32R)
            nc.sync.dma_start(out=wfc_sb[:], in_=wfc.rearrange("(c p) e -> p c e", p=128))
            # ---------------- P2: per head-pair attention ----------------
            if stage >= 2:
              with (
                tc.tile_pool(name="p2qk", bufs=2) as p2qk,
                tc.tile_pool(name="p2a", bufs=2) as p2a,
                tc.tile_pool(name="p2b", bufs=2) as p2b,
                tc.tile_pool(name="psA2", bufs=3, space="PSUM") as psA2,
                tc.tile_pool(name="psB", bufs=2, space="PSUM") as psB,
            ):
                for p in range(NP):
                    qpt = p2qk.tile([128, S], F32R, tag="qpt")
                    kpt = p2qk.tile([128, S], F32R, tag="kpt")
                    for wsb, dst in ((wq2_sb, qpt), (wk2_sb, kpt)):
                        tsb = t1q_sb if dst is qpt else t1k_sb
                        for blk in range(4):
                            pq = psA2.tile([128, 512], F32, tag="A2", name="pq")
                            nc.tensor.matmul(
                                pq[:],
                                wsb[:, p * 128 : (p + 1) * 128],
                                tsb[:, blk * 512 : (blk + 1) * 512],
                                start=True, stop=True,
                            )
                            nc.scalar.copy(out=dst[:, blk * 512 : (blk + 1) * 512], in_=pq[:])

                    if p2sub < 2:
                        continue
                    # stage 1: scores1[d2, s] = blockdiag(A).T-style product
                    attn1 = p2a.tile([128, S], F32, tag="attn1")
                    zpart = p2a.tile([128, 4], F32, tag="zpart")
                    for blk in range(4):
                        ps1 = psA2.tile([128, 512], F32, tag="A2")
                        nc.tensor.matmul(
                            ps1[:],
                            abd_sb[:],
                            kpt[:, blk * 512 : (blk + 1) * 512],
                            start=True, stop=True,
                        )
                        nc.scalar.activation(
                            out=attn1[:, blk * 512 : (blk + 1) * 512],
                            in_=ps1[:],
                            func=AF.Exp,
                            scale=0.125,
                            accum_out=zpart[:, blk : blk + 1],
                        )
                    z1 = p2a.tile([128, 1], F32, tag="z1")
                    nc.vector.reduce_sum(z1[:], zpart[:], axis=mybir.AxisListType.X)
                    nc.vector.reciprocal(z1[:], z1[:])
                    attn1n = attn1
                    nc.vector.tensor_scalar_mul(attn1n[:], attn1[:], z1[:])
                    nc.sync.dma_start(
                        out=attn[2 * p : 2 * p + 2].rearrange("h d s -> (h d) s"),
                        in_=attn1n[:],
                    )
                    if p2sub < 3:
                        continue
                    # transpose attn1n -> [s, d2] chunks (bf16); VV accumulation
                    attn1b = p2a.tile([128, S], BF16, tag="attn1b")
                    nc.vector.tensor_copy(out=attn1b[:], in_=attn1n[:])
                    attn1t = p2a.tile([128, SC, 128], BF16, tag="attn1t")
                    for g in range(4):
                        ptb_f = psA2.tile([128, 512], F32, tag="A2", name="ptb")
                        ptb = ptb_f.bitcast(BF16)[:, 0:512]
                        for c in range(4):
                            ch = g * 4 + c
                            nc.tensor.transpose(
                                ptb[:, c * 128 : (c + 1) * 128],
                                attn1b[:, ch * 128 : (ch + 1) * 128],
                                ident_bf[:],
                            )
                        nc.scalar.copy(
                            out=attn1t[:, g * 4 : (g + 1) * 4, :].rearrange("p c f -> p (c f)"),
                            in_=ptb[:],
                        )
                    pvv = psA2.tile([128, 512], F32, tag="A2")
                    for c in range(SC):
                        nc.tensor.matmul(
                            pvv[:, 0:128],
                            attn1t[:, c, :],
                            vp_sb[:, c, p * 128 : (p + 1) * 128],
                            start=(c == 0),
                            stop=(c == SC - 1),
                        )
                    vv = p2a.tile([128, 128], BF16, tag="vv")
                    # zero via DVE (f32r output = legal "rounded" producer)
                    nc.vector.tensor_scalar_mul(vv[:], ident[:], 0.0)
                    nc.scalar.copy(out=vv[0:64, 0:64], in_=pvv[0:64, 0:64])
                    nc.scalar.copy(out=vv[64:128, 64:128], in_=pvv[64:128, 64:128])

                    if p2sub < 4:
                        continue
                    # stage 2, groups of 8 s-chunks
                    vvv = p2b.tile([128, SC, 128], F32, tag="vvv")
                    for g in range(2):
                        G = 8
                        ps2 = psB.tile([128, 1024], F32, tag="B")
                        for c in range(G):
                            ch = g * G + c
                            nc.tensor.matmul(
                                ps2[:, c * 128 : (c + 1) * 128],
                                qpt[:, ch * 128 : (ch + 1) * 128],
                                atbd_sb[:],
                                start=True, stop=True,
                            )
                        ps2v = ps2.rearrange("p (c h f) -> p (c h) f", h=2, f=64)
                        m2 = p2b.tile([128, G * 2], F32, tag="m2")
                        nc.vector.reduce_max(m2[:], ps2v, axis=mybir.AxisListType.X)
                        nc.vector.tensor_tensor(
                            ps2v,
                            ps2v,
                            m2[:, :, None].to_broadcast([128, G * 2, 64]),
                            ALU.subtract,
                        )
                        exps2 = p2b.tile([128, G, 128], BF16, tag="exps2")
                        nc.scalar.activation(
                            out=exps2.rearrange("p c f -> p (c f)"),
                            in_=ps2.rearrange("p f -> p f"),
                            func=AF.Exp,
                            scale=0.125,
                        )
                        z2 = p2b.tile([128, G * 2], F32, tag="z2")
                        nc.vector.reduce_sum(
                            z2[:],
                            exps2.rearrange("p c (h f) -> p (c h) f", h=2, f=64),
                            axis=mybir.AxisListType.X,
                        )
                        nc.vector.reciprocal(z2[:], z2[:])
                        if p2sub < 5:
                            continue
                        att2t = p2b.tile([128, G, 128], BF16, tag="att2t")
                        for gg in range(2):
                            pt2_f = psA2.tile([128, 512], F32, tag="A2", name="pt2")
                            pt2 = pt2_f.bitcast(BF16)[:, 0:512]
                            for c in range(4):
                                ch = gg * 4 + c
                                nc.tensor.transpose(
                                    pt2[:, c * 128 : (c + 1) * 128],
                                    exps2[:, ch, :],
                                    ident_bf[:],
                                )
                            nc.scalar.copy(
                                out=att2t[:, gg * 4 : (gg + 1) * 4, :].rearrange("p c f -> p (c f)"),
                                in_=pt2[:],
                            )
                        if p2sub < 6:
                            continue
                        pvvv = psB.tile([128, 1024], F32, tag="B")
                        for c in range(G):
                            nc.tensor.matmul(
                                pvvv[:, c * 128 : (c + 1) * 128],
                                att2t[:, c, :],
                                vv[:],
                                start=True, stop=True,
                            )
                        nc.vector.tensor_tensor(
                            vvv[:, g * G : (g + 1) * G, :].rearrange(
                                "p c (h f) -> p (c h) f", h=2, f=64
                            ),
                            pvvv.rearrange("p (c h f) -> p (c h) f", h=2, f=64),
                            z2[:, :, None].to_broadcast([128, G * 2, 64]),
                            ALU.mult,
                        )
                    if p2sub < 7:
                        continue
                    # scatter-accumulate with the torch-reshape AP: element
                    # (part=ph*16+pl, chunk c, hh, f) -> row c*8+ph, col pl*64+f
                    for hh in range(2):
                        scrw = scratch_h[2 * p + hh].rearrange(
                            "(c ph) (pl f) -> ph pl c f", c=16, ph=8, pl=16, f=64
                        )
                        nc.gpsimd.dma_start(
                            out=scrw,
                            in_=vvv[:, :, hh * 64 : (hh + 1) * 64],
                            accum_op=ALU.add,
                        )

            # ---------------- P3: reshape + LN + fc + LN ----------------
            if stage >= 3:
              with (
                tc.tile_pool(name="p3", bufs=4) as p3,
                tc.tile_pool(name="p3s", bufs=3) as p3s,
                tc.tile_pool(name="psA3", bufs=8, space="PSUM") as psA3,
            ):
                for t in range(SC):
                    x1 = p3.tile([128, D], F32, tag="x1")
                    nc.sync.dma_start(out=x1[:], in_=scratch_h[t][:])

                    def layer_norm(dst, src, dst_dt):
                        stats = p3s.tile([128, 2, 6], F32, tag="stats")
                        srcv = src.rearrange("p (n f) -> p n f", f=512)
                        nc.vector.bn_stats(out=stats[:, 0, :], in_=srcv[:, 0, :])
                        nc.vector.bn_stats(out=stats[:, 1, :], in_=srcv[:, 1, :])
                        mv = p3s.tile([128, 2], F32, tag="mv")
                        nc.vector.bn_aggr(out=mv[:], in_=stats[:])
                        sd = p3s.tile([128, 1], F32, tag="sd")
                        nc.scalar.activation(
                            out=sd[:], in_=mv[:, 1:2], func=AF.Sqrt, bias=eps_sb[:]
                        )
                        nc.vector.reciprocal(sd[:], sd[:])
                        nmr = p3s.tile([128, 1], F32, tag="nmr")
                        nc.vector.tensor_tensor(nmr[:], mv[:, 0:1], sd[:], ALU.mult)
                        nc.vector.tensor_scalar_mul(nmr[:], nmr[:], -1.0)
                        nc.scalar.activation(
                            out=dst, in_=src, func=AF.Identity, bias=nmr[:], scale=sd[:]
                        )
                        if apply_gb:
                            nc.vector.tensor_tensor(dst, dst, g_sb[:], ALU.mult)
                            nc.vector.tensor_tensor(dst, dst, b_sb[:], ALU.add)

                    o1_dt = F32 if apply_gb else F32R
                    o1 = p3.tile([128, D], o1_dt, tag="o1")
                    layer_norm(o1[:], x1[:], o1_dt)
                    o1t = p3.tile([128, DC, 128], F32R, tag="o1t")
                    t_id = ident if apply_gb else ident_r
                    for g in range(2):
                        pt3_f = psA3.tile([128, 512], F32, tag="A3", name="pt3")
                        pt3 = pt3_f if apply_gb else pt3_f.bitcast(F32R)
                        for c in range(4):
                            dc = g * 4 + c
                            nc.tensor.transpose(
                                pt3[:, c * 128 : (c + 1) * 128],
                                o1[:, dc * 128 : (dc + 1) * 128],
                                t_id[:],
                            )
                        nc.scalar.copy(
                            out=o1t[:, g * 4 : (g + 1) * 4, :].rearrange("p c f -> p (c f)"),
                            in_=pt3[:],
                        )
                    pf0 = psA3.tile([128, 512], F32, tag="A3", name="pf0")
                    pf1 = psA3.tile([128, 512], F32, tag="A3", name="pf1")
                    for dc in range(DC):
                        for pf, fcc in ((pf0, 0), (pf1, 1)):
                            nc.tensor.matmul(
                                pf[:],
                                o1t[:, dc, :],
                                wfc_sb[:, dc, fcc * 512 : (fcc + 1) * 512],
                                start=(dc == 0),
                                stop=(dc == DC - 1),
                            )
                    ot = p3.tile([128, D], F32, tag="ot")
                    stats2 = p3s.tile([128, 2, 6], F32, tag="stats2")
                    nc.vector.bn_stats(out=stats2[:, 0, :], in_=pf0[:])
                    nc.vector.bn_stats(out=stats2[:, 1, :], in_=pf1[:])
                    mv2 = p3s.tile([128, 2], F32, tag="mv2")
                    nc.vector.bn_aggr(out=mv2[:], in_=stats2[:])
                    sd2 = p3s.tile([128, 1], F32, tag="sd2")
                    nc.scalar.activation(
                        out=sd2[:], in_=mv2[:, 1:2], func=AF.Sqrt, bias=eps_sb[:]
                    )
                    nc.vector.reciprocal(sd2[:], sd2[:])
                    nmr2 = p3s.tile([128, 1], F32, tag="nmr2")
                    nc.vector.tensor_tensor(nmr2[:], mv2[:, 0:1], sd2[:], ALU.mult)
                    nc.vector.tensor_scalar_mul(nmr2[:], nmr2[:], -1.0)
                    for pf, fcc in ((pf0, 0), (pf1, 1)):
                        nc.scalar.activation(
                            out=ot[:, fcc * 512 : (fcc + 1) * 512],
                            in_=pf[:],
                            func=AF.Identity,
                            bias=nmr2[:],
                            scale=sd2[:],
                        )
                    if apply_gb:
                        nc.vector.tensor_tensor(ot[:], ot[:], g_sb[:], ALU.mult)
                        nc.vector.tensor_tensor(ot[:], ot[:], b_sb[:], ALU.add)
                    nc.sync.dma_start(out=out[t * 128 : (t + 1) * 128, :], in_=ot[:])

    if legalize:
        _split_excess_waits(nc)
    return nc


_CACHE = {}


def _get_nc(apply_gb):
    if apply_gb not in _CACHE:
        _CACHE[apply_gb] = _build(apply_gb)
    return _CACHE[apply_gb]


last_results = None


def kernel(input_Q, input_K, input_V, Wq1, Wq2, Wk1, Wk2, Wv, Wfc, A, ln_g, ln_b):
    global last_results
    input_Q = np.ascontiguousarray(np.asarray(input_Q, dtype=np.float32))
    input_K = np.ascontiguousarray(np.asarray(input_K, dtype=np.float32))
    input_V = np.ascontiguousarray(np.asarray(input_V, dtype=np.float32))
    A = np.asarray(A, dtype=np.float32)
    ln_g = np.asarray(ln_g, dtype=np.float32)
    ln_b = np.asarray(ln_b, dtype=np.float32)

    abd = np.zeros((128, 128), dtype=np.float32)
    abd[0:64, 0:64] = A
    abd[64:128, 64:128] = A
    atbd = np.zeros((128, 128), dtype=np.float32)
    atbd[0:64, 0:64] = A.T
    atbd[64:128, 64:128] = A.T
    wfcp = (np.asarray(Wfc, dtype=np.float32)
            + np.eye(D, dtype=np.float32)).astype(np.float32)

    apply_gb = not (np.all(ln_g == 1.0) and np.all(ln_b == 0.0))
    nc = _get_nc(apply_gb)

    import ml_dtypes
    shared = {
        "wq1": np.ascontiguousarray(np.asarray(Wq1, dtype=np.float32).astype(ml_dtypes.bfloat16)),
        "wq2": np.ascontiguousarray(Wq2, dtype=np.float32),
        "wk1": np.ascontiguousarray(np.asarray(Wk1, dtype=np.float32).astype(ml_dtypes.bfloat16)),
        "wk2": np.ascontiguousarray(Wk2, dtype=np.float32),
        "wv": np.ascontiguousarray(np.asarray(Wv, dtype=np.float32).astype(ml_dtypes.bfloat16)),
        "wfc": wfcp,
        "abd": abd,
        "atbd": atbd,
        "lng": ln_g,
        "lnb": ln_b,
    }
    in_maps = []
    for b in range(B):
        m = dict(shared)
        m["xq"] = input_Q[b]
        m["xq_bf"] = np.ascontiguousarray(input_Q[b].astype(ml_dtypes.bfloat16))
        m["xk_bf"] = np.ascontiguousarray(input_K[b].astype(ml_dtypes.bfloat16))
        m["xv_bf"] = np.ascontiguousarray(input_V[b].astype(ml_dtypes.bfloat16))
        in_maps.append(m)

    trace = os.environ.get("KERNEL_TRACE") not in (None, "", "0")
    kwargs = {}
    if trace:
        import tempfile
        kwargs = dict(trace=True, tmpdir=tempfile.mkdtemp())
    res = run_bass_kernel_spmd(nc, in_maps, core_ids=list(range(B)), **kwargs)
    last_results = res
    outputs = np.stack([r["out"] for r in res.results])
    attn_vv = np.stack([r["attn"] for r in res.results])
    return outputs, attn_vv
